# revision 2
# baseline (speedup 1.0000x reference)
"""Trainium2 Bass kernel for nn_ClassicalEncoderDecoder.

The reference applies 8 sequential "rings" of 1024 Givens rotations each
(4 encoder + 4 decoder), with a sigmoid-gated blend in the middle:

    b          = x @ E                      (E = enc ring composite)
    bottleneck = (1-w) * b + w * hs
    out        = bottleneck @ D             (D = dec ring composite)

Everything is linear in x, so the whole computation collapses to two
matmuls with a rank-1 bias:

    bottleneck = x @ [(1-w) E]      + w * hs
    out        = x @ [(1-w) E D]    + w * (hs @ D)

The composite matrices are built on host from the tiny angle params via a
closed-form per-diagonal construction of each ring's rotation product
(each ring matrix is effectively banded for generic angles), then composed
with a handful of 1024^2 BLAS matmuls.

This session's devices are axon-tunneled NeuronCores: the host<->device
link runs at only ~40 MB/s (measured), so end-to-end latency is dominated
by transferred bytes, not device compute.  The design therefore:

  * keeps ONE compiled executable cached across kernel() calls (the stock
    run_bass_kernel_spmd re-traces, re-lowers and re-compiles the jit on
    every call);
  * ships x in bf16 [8192, 1024] exactly as laid out on host (the device
    transposes via the DMA xbar);
  * ships W once, row-sharded across the 8 cores (512 KB/core), and
    reassembles it on-device with an HBM AllGather over NeuronLink;
  * returns both outputs in bf16;
  * overlaps the host-side composite build with the async x upload.
"""

import math
from contextlib import ExitStack

import numpy as np

import jax
from jax.experimental.shard_map import shard_map
from jax.sharding import Mesh, NamedSharding, PartitionSpec

import concourse.bass as bass
import concourse.mybir as mybir
import concourse.tile as tile
from concourse import bass2jax
from concourse.bass2jax import _bass_exec_p, install_neuronx_cc_hook
from concourse.vector_clock import ScopedClock

N_CORES = 8
B_FULL = 8192
NF = 1024            # feature dim
BS = B_FULL // N_CORES   # batch rows per core
NOUT = 2 * NF        # concat of [bottleneck | out] feature columns
F32 = mybir.dt.float32
BF16 = mybir.dt.bfloat16
NP_BF16 = mybir.dt.np(BF16)

# ---------------------------------------------------------------------------
# Tile/walrus workaround: this container's walrus rejects instructions that
# carry more than one semaphore wait ("Too many sync wait commands").  Tile's
# add_semaphores freely attaches several waits to one instruction, so after
# scheduling we split extra waits onto single-wait nops placed immediately
# before the instruction on the same engine.
# ---------------------------------------------------------------------------

_TILE_PSEUDO_CLASSES = tuple(
    c
    for c in (
        getattr(tile, "BassTileRelease", None),
        getattr(tile, "BassTileCriticalSection", None),
        getattr(tile, "TileBranchInst", None),
        getattr(tile, "BassTileLoopBlock", None),
        getattr(tile, "BassTileBranchHintPlaceholder", None),
    )
    if c is not None
)


def _split_excess_waits(nc, insts):
    out = []
    for inst in insts:
        si = getattr(inst, "sync_info", None)
        waits = list(si.on_wait) if si is not None else []
        eng = getattr(inst, "engine", None)
        if (
            len(waits) > 1
            and not isinstance(inst, _TILE_PSEUDO_CLASSES)
            and eng is not None
            and eng != mybir.EngineType.Unassigned
        ):
            for w in waits[:-1]:
                out.append(
                    mybir.InstNoOp(
                        name=nc.get_next_instruction_name(),
                        ins=[],
                        outs=[],
                        engine=eng,
                        sync_info=mybir.SyncInfo(on_wait=[w], on_update=[]),
                        bass_nofuse=True,
                    )
                )
            inst.sync_info = mybir.SyncInfo(
                on_wait=[waits[-1]], on_update=list(si.on_update)
            )
        out.append(inst)
    return out


_ORIG_LOWER_ORDERED = tile.TileContext._lower_ordered_insts


def _patched_lower_ordered_insts(self, ordered):
    for bb_name in list(ordered.keys()):
        ordered[bb_name] = _split_excess_waits(self.nc, ordered[bb_name])
    return _ORIG_LOWER_ORDERED(self, ordered)


if getattr(tile.TileContext._lower_ordered_insts, "__name__", "") != "_patched_lower_ordered_insts":
    tile.TileContext._lower_ordered_insts = _patched_lower_ordered_insts


def _patched_drain_and_barrier(self, tick_clock, wait_clock):
    nc = self.nc
    probe = nc.sync.nop(nofuse=True)
    wait_clock.add_sem_waits(probe.ins, ScopedClock({None: tick_clock.global_clock}))
    si = probe.ins.sync_info
    waits = list(si.on_wait) if si is not None else []
    if len(waits) > 1:
        probe.ins.sync_info = mybir.SyncInfo(on_wait=[waits[0]], on_update=[])
        for w in waits[1:]:
            n = nc.sync.nop(nofuse=True)
            n.ins.sync_info = mybir.SyncInfo(on_wait=[w], on_update=[])
    nc.sync.drain()
    nc.all_engine_barrier()
    popped = nc._tile_sem_poison_stack.pop()
    assert popped is self._sem_poison
    nc.clear_and_free_semaphores(list(self.sems.allocated().values()))
    nc.all_engine_barrier()


if getattr(tile.TileContext._drain_and_barrier, "__name__", "") != "_patched_drain_and_barrier":
    tile.TileContext._drain_and_barrier = _patched_drain_and_barrier


# ---------------------------------------------------------------------------
# Host-side composite-rotation precompute (closed form, vectorized)
# ---------------------------------------------------------------------------


def _ring_M(angles: np.ndarray, thresh: float = 1e-14) -> np.ndarray:
    """Closed-form composite M of one ring such that
    _ring_T_inplace(XT) == M @ XT, i.e. apply_ring(x) == x @ M^T.

    M = Q @ R_{n-1} with Q = R_0 R_1 ... R_{n-2} (adjacent planes (k,k+1))
    and R_{n-1} acting on the wraparound plane (n-1, 0).  Q is assembled
    per-diagonal:
      Q[j+1, j] = s_j
      Q[i, j]   = (-1)^(j-i) ct_i (prod_{m=i}^{j-1} s_m) c_j  (i<=j<=n-2)
      Q[i, n-1] = (-1)^(n-1-i) ct_i prod_{m=i}^{n-2} s_m
    with ct_i = c_{i-1} (ct_0 = 1).  Diagonals decay geometrically in the
    sine products, so the loop stops once they fall below `thresh`
    (adversarial angle vectors just run all n diagonals).
    """
    n = angles.shape[0]
    c = np.cos(angles.astype(np.float64))
    s = np.sin(angles.astype(np.float64))
    ct = np.ones(n)
    ct[1:] = c[:-1]

    Q = np.zeros((n, n))
    Q[np.arange(1, n), np.arange(0, n - 1)] = s[: n - 1]
    S = ct.copy()
    sign = 1.0
    for d in range(0, n):
        i = np.arange(0, n - d)
        j = i + d
        vals = sign * S[: n - d]
        inner = j <= n - 2
        Q[i[inner], j[inner]] = vals[inner] * c[j[inner]]
        if not inner.all():
            Q[i[~inner], j[~inner]] = vals[~inner]
        if d < n - 1:
            S = S[: n - d - 1] * s[d : n - 1]
            if np.max(np.abs(S)) < thresh:
                break
        sign = -sign
    M = Q
    col0 = Q[:, 0].copy()
    coln = Q[:, n - 1].copy()
    M[:, 0] = c[n - 1] * col0 - s[n - 1] * coln
    M[:, n - 1] = s[n - 1] * col0 + c[n - 1] * coln
    return M


def _host_params(angles_enc, angles_dec, hidden_weight, hidden_state):
    """Build W [NF, 2*NF] and bias [2*NF] (both float32).

    apply_ring chain: b = x @ M_e0^T M_e1^T ... so E^T = M_e3 M_e2 M_e1 M_e0.
    """
    n = NF
    Me = [_ring_M(angles_enc[b]).astype(np.float32) for b in range(angles_enc.shape[0])]
    Md = [_ring_M(angles_dec[b]).astype(np.float32) for b in range(angles_dec.shape[0])]
    ET = (Me[3] @ Me[2]) @ (Me[1] @ Me[0])
    DT = (Md[3] @ Md[2]) @ (Md[1] @ Md[0])
    EDT = DT @ ET
    hs64 = hidden_state.astype(np.float64)
    dhs = DT.astype(np.float64) @ hs64
    w = 1.0 / (1.0 + np.exp(-np.float64(hidden_weight[0])))
    W = np.empty((n, NOUT), np.float32)
    W[:, :n] = (1.0 - w) * ET.T
    W[:, n:] = (1.0 - w) * EDT.T
    bias = np.concatenate([w * hs64, w * dhs]).astype(np.float32)
    return W, bias


# ---------------------------------------------------------------------------
# Device program (per-core, SPMD over 8 cores)
# ---------------------------------------------------------------------------


def _build_program():
    nc = bass.Bass(trn_type="TRN2", num_devices=N_CORES)
    xin = nc.dram_tensor("xin", [BS, NF], BF16, kind="ExternalInput")
    wsl = nc.dram_tensor("wsl", [NF // N_CORES, NOUT], BF16, kind="ExternalInput")
    bvc = nc.dram_tensor("bvc", [NOUT], F32, kind="ExternalInput")
    bot = nc.dram_tensor("bot", [BS, NF], BF16, kind="ExternalOutput")
    outp = nc.dram_tensor("outp", [BS, NF], BF16, kind="ExternalOutput")

    KT = NF // 128    # 8 contraction (feature) tiles
    MT = BS // 128    # 8 batch row tiles
    NT = NOUT // 512  # 4 moving-dim column chunks

    with tile.TileContext(nc) as tc, ExitStack() as ctx:
        dram = ctx.enter_context(tc.tile_pool(name="dram", bufs=1, space="DRAM"))
        const = ctx.enter_context(tc.tile_pool(name="const", bufs=1))
        psum = ctx.enter_context(tc.tile_pool(name="psum", bufs=2, space="PSUM"))
        outp_pool = ctx.enter_context(tc.tile_pool(name="outp", bufs=3))

        # --- W: 1/8 slice arrives per core; AllGather over NeuronLink. ---
        w_bounce = dram.tile([NF // N_CORES, NOUT], BF16)
        w_full = dram.tile([NF, NOUT], BF16)
        nc.gpsimd.dma_start(w_bounce[:], wsl[:])
        nc.gpsimd.collective_compute(
            "AllGather",
            mybir.AluOpType.bypass,
            replica_groups=[list(range(N_CORES))],
            ins=[w_bounce[:].opt()],
            outs=[w_full[:].opt()],
        )
        w_k = []
        for k in range(KT):
            wk = const.tile([128, NOUT], BF16, tag=f"w{k}")
            nc.sync.dma_start(wk[:], w_full[k * 128:(k + 1) * 128, :])
            w_k.append(wk)

        # --- x: transpose into feature-major tiles via the DMA xbar. ---
        xt_k = []
        for k in range(KT):
            xk = const.tile([128, BS], BF16, tag=f"xt{k}")
            nc.sync.dma_start_transpose(xk[:], xin[:, k * 128:(k + 1) * 128])
            xt_k.append(xk)

        # --- bias broadcast to all 128 partitions. ---
        b_sb = const.tile([128, NOUT], F32)
        bvap = bvc[:]
        nc.gpsimd.dma_start(
            out=b_sb[:],
            in_=bass.AP(tensor=bvap.tensor, offset=bvap.offset, ap=[[0, 128]] + list(bvap.ap)),
        )

        for m in range(MT):
            ps = psum.tile([128, NOUT], F32)
            prev_mm = [None] * NT
            for k in range(KT):
                lhs = xt_k[k][:, m * 128:(m + 1) * 128]
                for n4 in range(NT):
                    rhs = w_k[k][:, n4 * 512:(n4 + 1) * 512]
                    mm = nc.tensor.matmul(
                        ps[:, n4 * 512:(n4 + 1) * 512],
                        lhs,
                        rhs,
                        start=(k == 0),
                        stop=(k == KT - 1),
                    )
                    if prev_mm[n4] is not None:
                        # Pin in-group accumulation order (PE executes in
                        # issue order, so a scheduling-only dep suffices;
                        # a reordered start=True matmul would zero earlier
                        # partials).
                        tile.add_dep_helper(
                            mm.ins,
                            prev_mm[n4].ins,
                            sync=False,
                            reason="psum accumulation k-order",
                        )
                    prev_mm[n4] = mm
            o = outp_pool.tile([128, NOUT], BF16)
            nc.vector.tensor_add(o[:], ps[:], b_sb[:])
            nc.sync.dma_start(bot[m * 128:(m + 1) * 128, :], o[:, :NF])
            nc.sync.dma_start(outp[m * 128:(m + 1) * 128, :], o[:, NF:])
    return nc


# ---------------------------------------------------------------------------
# Cached PJRT runner (the per-call portion of bass2jax.run_bass_via_pjrt,
# with the trace/lower/compile hoisted out of the per-call path).
# ---------------------------------------------------------------------------

_RUNNER = None


def _make_runner():
    install_neuronx_cc_hook()
    nc = _build_program()

    in_names, out_names, out_avals = [], [], []
    partition_name = nc.partition_id_tensor.name if nc.partition_id_tensor else None
    for alloc in nc.m.functions[0].allocations:
        if not isinstance(alloc, mybir.MemoryLocationSet):
            continue
        name = alloc.memorylocations[0].name
        if alloc.kind == "ExternalInput":
            if name != partition_name:
                in_names.append(name)
        elif alloc.kind == "ExternalOutput":
            out_names.append(name)
            out_avals.append(
                jax.core.ShapedArray(
                    tuple(alloc.tensor_shape), mybir.dt.np(alloc.dtype)
                )
            )
    all_in_names = list(in_names)
    if partition_name is not None:
        all_in_names.append(partition_name)

    def _body(*args):
        operands = list(args)
        if partition_name is not None:
            operands.append(bass2jax.partition_id_tensor())
        outs = _bass_exec_p.bind(
            *operands,
            out_avals=tuple(out_avals),
            in_names=tuple(all_in_names),
            out_names=tuple(out_names),
            lowering_input_output_aliases=(),
            sim_require_finite=True,
            sim_require_nnan=True,
            nc=nc,
        )
        return tuple(outs)

    devices = jax.devices()[:N_CORES]
    mesh = Mesh(np.asarray(devices), ("core",))
    spec = PartitionSpec("core")
    fn = jax.jit(
        shard_map(
            _body,
            mesh=mesh,
            in_specs=(spec,) * len(in_names),
            out_specs=(spec,) * len(out_names),
            check_rep=False,
        )
    )
    x_sharding = NamedSharding(mesh, spec)
    return fn, x_sharding


def _get_runner():
    global _RUNNER
    if _RUNNER is None:
        _RUNNER = _make_runner()
    return _RUNNER


# ---------------------------------------------------------------------------
# Entry point
# ---------------------------------------------------------------------------


def kernel(x, angles_enc, angles_dec, hidden_weight, hidden_state):
    fn, x_sharding = _get_runner()

    # Start the (slow, ~40 MB/s) x upload first; it streams while the host
    # builds the composite weights below.
    xb = np.asarray(x, np.float32).astype(NP_BF16)
    x_dev = jax.device_put(xb, x_sharding)

    W, bias = _host_params(
        np.asarray(angles_enc, np.float32),
        np.asarray(angles_dec, np.float32),
        np.asarray(hidden_weight, np.float32),
        np.asarray(hidden_state, np.float32),
    )
    Wb = W.astype(NP_BF16)              # [1024, 2048]; sharded 128 rows/core
    bias8 = np.tile(bias, N_CORES)      # [8*2048]; per-core slice = full bias

    bot_d, out_d = fn(x_dev, Wb, bias8)

    bottleneck = np.asarray(bot_d).astype(np.float32)
    out = np.asarray(out_d).astype(np.float32)
    return bottleneck, out
